# revision 18
# baseline (speedup 1.0000x reference)
"""DeepseekV3 top-k router (moe_routing) on 8 Trainium2 NeuronCores.

Sharding (hardcoded from the problem spec):
  - Data-parallel over the token dim: 8192 tokens -> 8 shards of 1024.
  - Router weight [256, 7168] and bias [256] replicated to every core.

Numerics: logits = x.w need ~fp32 precision for the top-k indices to
match the fp32 reference exactly. Decompose x = xh + xl (fp16 hi +
residual), w = wh + wl, and compute
    logits = xh.wh  +  2^-18 * (fp8(xh).fp8(wl*2^18) + fp8(xl*2^12).fp8(wh*2^6))
with both correction terms in ONE fp8 DoubleRow matmul per k-tile
(DoubleRow contracts [K,2,M] x [K,2,N] over K and the 2-subtile dim;
measured on HW it runs at 1 cycle/output-row, its win is carrying two
fp8 products per instruction). Verified on the fixed eval inputs:
0/65536 index flips vs the fp32 reference, max weight rel err 1.3e-5.

Matmul orientation: w stationary / x moving, so each instruction covers
a whole token group (the PE re-issues LDWEIGHTS per matmul; small
instructions are dispatch/LDW-bound at ~109-135ns):
  - hh:  [K=128,128e].T @ [K=128,Nt] fp16
  - corr:[K,2,128e].T @ [K,2,Nt] fp8 DoubleRow
Logits accumulate transposed [expert, token]; hh+corr combine on
ScalarE+DVE, then 128x128 tiles transpose back via PE-transpose.

Schedule: token groups [256,256,256,128,128] with double-buffered x
tiles (no group-boundary DMA bubbles); the two trailing 128-token
groups keep the end-of-kernel epilogue tail short. Rings are static:
sync carries xh + w16, scalar carries xl8 + w8 + outputs. Every group
is chunked along k so matmuls track chunk arrival; fp8(xh) is cast
on-device on the DVE; each group's epilogue is emitted one group late
so the next group's casts never queue behind it on the DVE.
"""

import os
import sys

for _p in ("/opt/trn_rl_repo", "/root/.axon_site/_ro/trn_rl_repo"):
    if os.path.isdir(_p) and _p not in sys.path:
        sys.path.append(_p)

from contextlib import ExitStack

import numpy as np
import ml_dtypes

import concourse.bass as bass
import concourse.bacc as bacc
import concourse.mybir as mybir
import concourse.tile as tile
from concourse import masks

N_CORES = 8
T_FULL = 8192
HIDDEN = 7168
N_EXPERTS = 256
TOP_K = 8
N_GROUP = 8
TOPK_GROUP = 4
SCALING = 2.5

P = 128
GROUPS = [256, 256, 256, 128, 128]   # token groups per core (sum 1024)
GMAX = 256
F32 = mybir.dt.float32
F16 = mybir.dt.float16
F8 = mybir.dt.float8e4
E4NP = ml_dtypes.float8_e4m3
S_XL = 2.0 ** 12              # xl8 = fp8(xl * S_XL)
S_WL = 2.0 ** 18              # wl8 = fp8(wl * S_WL)
S_WH = 2.0 ** 6               # wh8 = fp8(wh * S_WH)
S_CORR = S_XL * S_WH          # both fp8 products land at this scale
WARMUP_MMS = 120


def build_module(t_shard=T_FULL // N_CORES, hidden=HIDDEN):
    """Build + compile the per-core Bass module (SPMD: same program, 8 cores)."""
    KT = hidden // P            # hidden k-tiles (56)
    TT = t_shard // P           # token tiles per core (8)
    E = N_EXPERTS
    EPG = E // N_GROUP          # experts per group (32)
    AX = mybir.AxisListType
    OP = mybir.AluOpType
    DRM = mybir.MatmulPerfMode.DoubleRow
    starts = [sum(GROUPS[:i]) for i in range(len(GROUPS))]

    nc = bacc.Bacc("TRN2", debug=False, target_bir_lowering=False)

    # x pre-tiled per group-size class so each (group, k-chunk) DMA is a
    # plain multi-dim slice with long contiguous runs
    NG_A = sum(1 for t in GROUPS if t == 256)
    NG_B = sum(1 for t in GROUPS if t == 128)
    xh_a = nc.dram_tensor("xh16_a", [NG_A, P, KT, 256], F16, kind="ExternalInput").ap()
    xl_a = nc.dram_tensor("xl8_a", [NG_A, P, KT, 256], F8, kind="ExternalInput").ap()
    xh_b = nc.dram_tensor("xh16_b", [max(NG_B, 1), P, KT, 128], F16, kind="ExternalInput").ap()
    xl_b = nc.dram_tensor("xl8_b", [max(NG_B, 1), P, KT, 128], F8, kind="ExternalInput").ap()
    w16_in = nc.dram_tensor("w16", [P, KT, E], F16, kind="ExternalInput").ap()
    w8_in = nc.dram_tensor("w8", [P, 2, KT, E], F8, kind="ExternalInput").ap()
    bias = nc.dram_tensor("bias", [E], F32, kind="ExternalInput").ap()
    out_i = nc.dram_tensor("topk_idx", [t_shard, TOP_K], mybir.dt.int32,
                           kind="ExternalOutput").ap()
    out_w = nc.dram_tensor("topk_w", [t_shard, TOP_K], F32,
                           kind="ExternalOutput").ap()
    sink = nc.dram_tensor("warm_sink", [P, 1], F32).ap()

    kcuts = [0, 6, 20, 36, KT]
    kranges = [(kcuts[i], kcuts[i + 1]) for i in range(len(kcuts) - 1)]

    with tile.TileContext(nc) as tc, ExitStack() as ctx:
        const = ctx.enter_context(tc.tile_pool(name="const", bufs=1))
        wpool = ctx.enter_context(tc.tile_pool(name="wres", bufs=1))
        xpool = ctx.enter_context(tc.tile_pool(name="xin", bufs=2))
        cpool = ctx.enter_context(tc.tile_pool(name="cmb", bufs=2))
        spool = ctx.enter_context(tc.tile_pool(name="scr", bufs=2))
        smalls = ctx.enter_context(tc.tile_pool(name="small", bufs=2))
        opool = ctx.enter_context(tc.tile_pool(name="outs", bufs=1))
        pshh = ctx.enter_context(tc.tile_pool(name="pshh", bufs=2, space="PSUM"))
        pscc = ctx.enter_context(tc.tile_pool(name="pscc", bufs=2, space="PSUM"))
        ptp = ctx.enter_context(tc.tile_pool(name="ptp", bufs=2, space="PSUM"))
        pswarm = ctx.enter_context(tc.tile_pool(name="psw", bufs=1, space="PSUM"))

        # ---- PE warm-up: keep the HAM clock-gate busy from t=0 ----
        wu = const.tile([P, 64], F16)
        nc.gpsimd.memset(wu[:], 0.0)
        psw = pswarm.tile([P, 64], F32)
        for _ in range(WARMUP_MMS):
            nc.tensor.matmul(psw[:64], wu[:], wu[:], start=True, stop=True)
        wsum = smalls.tile([P, 1], F32, tag="wsum")
        nc.vector.tensor_reduce(wsum[:], psw[:], axis=AX.X, op=OP.add)
        # SWDGE ring: must not block the HWDGE rings while warmup runs
        nc.gpsimd.dma_start(out=sink, in_=wsum[:])

        # ---- constants ----
        bias_bc = const.tile([P, E], F32)
        bias_src = bass.AP(tensor=bias.tensor, offset=0, ap=[[0, P], [1, E]])
        ident = const.tile([P, P], F32)
        masks.make_identity(nc, ident[:])

        # ---- resident w: fp16 hi + fp8 (wl, wh) pair ----
        w16_sb = wpool.tile([P, KT, E], F16)
        w8_sb = wpool.tile([P, 2, KT, E], F8)

        out_i_sb = opool.tile([P, TT, TOP_K], mybir.dt.int32)
        out_w_sb = opool.tile([P, TT, TOP_K], F32)

        def epilogue_tile(tt, ps_t):
            # sigmoid scores from the transposed [token, expert] PSUM tile
            s = spool.tile([P, E], F32, tag="s")
            nc.scalar.activation(s[:], ps_t[:, :E],
                                 mybir.ActivationFunctionType.Sigmoid)

            # scores for choice = sigmoid + bias
            sc = spool.tile([P, E], F32, tag="sc")
            nc.vector.tensor_tensor(sc[:], s[:], bias_bc[:], op=OP.add)

            sc_g = sc[:].rearrange("p (g c) -> p g c", c=EPG)

            # per-group top-2 sum
            gmax = smalls.tile([P, N_GROUP], F32, tag="gmax")
            nc.vector.tensor_reduce(gmax[:], sc_g, axis=AX.X, op=OP.max)
            rep = spool.tile([P, E], F32, tag="rep")
            nc.vector.match_replace(rep[:], gmax[:], sc[:], -1e30)
            gsec = smalls.tile([P, N_GROUP], F32, tag="gsec")
            nc.vector.tensor_reduce(gsec[:],
                                    rep[:].rearrange("p (g c) -> p g c", c=EPG),
                                    axis=AX.X, op=OP.max)
            gsum = smalls.tile([P, N_GROUP], F32, tag="gsum")
            nc.vector.tensor_tensor(gsum[:], gmax[:], gsec[:], op=OP.add)

            # top-4 groups: sort the 8 group scores, threshold at 4th
            gsort = smalls.tile([P, 8], F32, tag="gsort")
            nc.vector.max(gsort[:], gsum[:])
            gmask = smalls.tile([P, N_GROUP], F32, tag="gmask")
            nc.vector.tensor_scalar(gmask[:], gsum[:],
                                    gsort[:, TOPK_GROUP - 1:TOPK_GROUP], None,
                                    op0=OP.is_ge)

            # masked scores = sc * group_mask
            masked = spool.tile([P, E], F32, tag="masked")
            nc.vector.tensor_tensor(masked[:].rearrange("p (g c) -> p g c", c=EPG),
                                    sc_g,
                                    gmask[:].unsqueeze(2).broadcast_to(
                                        (P, N_GROUP, EPG)),
                                    op=OP.mult)

            # top-8 experts (desc values + indices, lax.top_k semantics)
            t8v = smalls.tile([P, TOP_K], F32, tag="t8v")
            nc.vector.max(t8v[:], masked[:])
            t8i = smalls.tile([P, TOP_K], mybir.dt.uint32, tag="t8i")
            nc.vector.max_index(t8i[:], t8v[:], masked[:])

            # output copy rides GpSimd so it stays off the DVE chain
            nc.gpsimd.tensor_copy(out_i_sb[:, tt, :], t8i[:])

            # gather sigmoid scores at the top-8 indices
            mr2 = spool.tile([P, E], F32, tag="mr2")
            nc.vector.match_replace(mr2[:], t8v[:], masked[:], -1.0)
            sel = spool.tile([P, E], F32, tag="sel")
            nc.vector.tensor_tensor(sel[:], mr2[:], masked[:], op=OP.not_equal)
            nc.vector.tensor_tensor(sel[:], sel[:], s[:], op=OP.mult)
            v8 = smalls.tile([P, TOP_K], F32, tag="v8")
            nc.vector.max(v8[:], sel[:])
            i8 = smalls.tile([P, TOP_K], mybir.dt.uint32, tag="i8")
            nc.vector.max_index(i8[:], v8[:], sel[:])
            # eqm[p, k, j] = (idx_choice[p, k] == idx_s[p, j]); sg = eqm @ v8
            eqm = smalls.tile([P, TOP_K, TOP_K], F32, tag="eqm")
            nc.vector.tensor_tensor(eqm[:],
                                    t8i[:].unsqueeze(2).broadcast_to(
                                        (P, TOP_K, TOP_K)),
                                    i8[:].unsqueeze(1).broadcast_to(
                                        (P, TOP_K, TOP_K)),
                                    op=OP.is_equal)
            nc.vector.tensor_tensor(eqm[:], eqm[:],
                                    v8[:].unsqueeze(1).broadcast_to(
                                        (P, TOP_K, TOP_K)),
                                    op=OP.mult)
            sg = smalls.tile([P, TOP_K], F32, tag="sg")
            nc.vector.tensor_reduce(sg[:], eqm[:], axis=AX.X, op=OP.add)

            # weights = sg / sum(sg) * SCALING
            den = smalls.tile([P, 1], F32, tag="den")
            nc.vector.tensor_reduce(den[:], sg[:], axis=AX.X, op=OP.add)
            rcp = smalls.tile([P, 1], F32, tag="rcp")
            nc.vector.reciprocal(rcp[:], den[:])
            nc.vector.tensor_scalar(out_w_sb[:, tt, :], sg[:], rcp[:, 0:1],
                                    SCALING, op0=OP.mult, op1=OP.mult)

        oi = out_i.rearrange("(t p) k -> p t k", p=P)
        ow = out_w.rearrange("(t p) k -> p t k", p=P)

        def epilogue_group(g, ps_h, ps_c):
            TBg = GROUPS[g]
            nt = TBg // P
            tt0 = starts[g] // P
            # combine halves: stage hh via ScalarE, stt on DVE
            sA = cpool.tile([P, 2, GMAX], F32, tag="sA")
            comb = cpool.tile([P, 2, GMAX], F32, tag="comb")
            for h in range(2):
                nc.scalar.activation(sA[:, h, :TBg], ps_h[:, h, :TBg],
                                     mybir.ActivationFunctionType.Copy)
                nc.vector.scalar_tensor_tensor(comb[:, h, :TBg], ps_c[:, h, :TBg],
                                               1.0 / S_CORR, sA[:, h, :TBg],
                                               op0=OP.mult, op1=OP.add)
            for t in range(nt):
                ps_t = ptp.tile([P, E], F32, tag="pst")
                for h in range(2):
                    nc.tensor.transpose(ps_t[:, h * P:(h + 1) * P],
                                        comb[:, h, t * P:(t + 1) * P],
                                        ident[:])
                epilogue_tile(tt0 + t, ps_t)
            nc.scalar.dma_start(out=oi[:, tt0:tt0 + nt],
                                in_=out_i_sb[:, tt0:tt0 + nt])
            nc.scalar.dma_start(out=ow[:, tt0:tt0 + nt],
                                in_=out_w_sb[:, tt0:tt0 + nt])

        pending = None
        for g, TBg in enumerate(GROUPS):
            t0 = starts[g]
            xh_t = xpool.tile([P, KT, GMAX], F16, tag="xh", name=f"xh_{g}")
            x8_t = xpool.tile([P, 2, KT, GMAX], F8, tag="x8", name=f"x8_{g}")
            gi = g if TBg == 256 else g - NG_A
            xh_src = xh_a if TBg == 256 else xh_b
            xl_src = xl_a if TBg == 256 else xl_b
            for (k0, k1) in kranges:
                nc.sync.dma_start(out=xh_t[:, k0:k1, :TBg],
                                  in_=xh_src[gi, :, k0:k1])
                nc.scalar.dma_start(out=x8_t[:, 1, k0:k1, :TBg],
                                    in_=xl_src[gi, :, k0:k1])
                if g == 0:
                    # w rides the first group's window: w16 on sync, w8 on
                    # scalar keeps both rings evenly loaded early
                    nc.sync.dma_start(out=w16_sb[:, k0:k1],
                                      in_=w16_in[:, k0:k1])
                    nc.scalar.dma_start(out=w8_sb[:, :, k0:k1],
                                        in_=w8_in[:, :, k0:k1])
                # on-device cast x8[:,0] = fp8(xh) on the DVE
                nc.vector.tensor_copy(x8_t[:, 0, k0:k1, :TBg],
                                      xh_t[:, k0:k1, :TBg])
            if g == 0:
                nc.scalar.dma_start(out=bias_bc[:], in_=bias_src)

            ps_h = pshh.tile([P, 2, GMAX], F32, tag="psh")
            ps_c = pscc.tile([P, 2, GMAX], F32, tag="psc")
            # each expert-half runs its FULL k sweep before the other half
    # touches the same PSUM bank: accumulation groups are per-bank, so
            # the halves' groups must be consecutive and closed, never
            # interleaved. Mode flips twice per group (hh runs, then DR).
            for h in range(2):
                for k in range(KT):
                    nc.tensor.matmul(ps_h[:, h, :TBg],
                                     w16_sb[:, k, h * P:(h + 1) * P],
                                     xh_t[:, k, :TBg],
                                     start=(k == 0), stop=(k == KT - 1))
            for h in range(2):
                for k in range(KT):
                    nc.tensor.matmul(ps_c[:, h, :TBg],
                                     w8_sb[:, :, k, h * P:(h + 1) * P],
                                     x8_t[:, :, k, :TBg],
                                     start=(k == 0), stop=(k == KT - 1),
                                     perf_mode=DRM)

            # emit the PREVIOUS group's epilogue now, so this one's casts
            # did not queue behind it on the DVE
            if pending is not None:
                epilogue_group(*pending)
            pending = (g, ps_h, ps_c)
        epilogue_group(*pending)

    nc.compile()
    return nc


_CACHED = {}


def _get_module():
    key = (T_FULL // N_CORES, HIDDEN)
    if key not in _CACHED:
        _CACHED[key] = build_module(*key)
    return _CACHED[key]


def _make_in_maps(x, weight, e_score_correction_bias):
    x = np.asarray(x, dtype=np.float32)
    w = np.asarray(weight, dtype=np.float32)
    b = np.ascontiguousarray(np.asarray(e_score_correction_bias, dtype=np.float32))
    hidden = x.shape[1]
    E = w.shape[0]
    KT = hidden // P

    wT = np.ascontiguousarray(w.T)                      # [H, E] f32
    w16 = wT.astype(np.float16)
    wl8 = ((wT - w16.astype(np.float32)) * np.float32(S_WL)).astype(E4NP)
    wh8 = (w16.astype(np.float32) * np.float32(S_WH)).astype(E4NP)

    def tile_w(a):                                      # [H, E] -> [P, KT, E]
        return np.ascontiguousarray(a.reshape(KT, P, E).transpose(1, 0, 2))

    w16_t = tile_w(w16)
    w8_t = np.ascontiguousarray(
        np.stack([tile_w(wl8), tile_w(wh8)], axis=1))   # [P, 2, KT, E]

    def tile_x(a):
        # [H, T] -> dict of per-group-size stacks [NG, P, KT, TBg]
        av, bv = [], []
        t0 = 0
        for TBg in GROUPS:
            v = np.ascontiguousarray(
                a[:, t0:t0 + TBg].reshape(KT, P, TBg).transpose(1, 0, 2))
            (av if TBg == 256 else bv).append(v)
            t0 += TBg
        out = {}
        out["a"] = np.ascontiguousarray(np.stack(av)) if av else None
        out["b"] = (np.ascontiguousarray(np.stack(bv)) if bv
                    else np.zeros((1, P, KT, 128), a.dtype))
        return out

    t_shard = x.shape[0] // N_CORES
    in_maps = []
    for i in range(N_CORES):
        shardT = np.ascontiguousarray(x[i * t_shard:(i + 1) * t_shard].T)
        xh = shardT.astype(np.float16)
        xl8 = ((shardT - xh.astype(np.float32))
               * np.float32(S_XL)).astype(E4NP)
        xhp, xlp = tile_x(xh), tile_x(xl8)
        in_maps.append({"xh16_a": xhp["a"], "xl8_a": xlp["a"],
                        "xh16_b": xhp["b"], "xl8_b": xlp["b"],
                        "w16": w16_t, "w8": w8_t, "bias": b})
    return in_maps


def run_hw(x, weight, e_score_correction_bias, trace=False, **kwargs):
    """Run on the 8 NeuronCores; returns ((idx, w), BassKernelResults)."""
    from concourse.bass_utils import run_bass_kernel_spmd

    nc = _get_module()
    in_maps = _make_in_maps(x, weight, e_score_correction_bias)
    res = run_bass_kernel_spmd(nc, in_maps, core_ids=list(range(N_CORES)),
                               trace=trace, **kwargs)
    idx = np.concatenate([r["topk_idx"] for r in res.results], axis=0)
    w = np.concatenate([r["topk_w"] for r in res.results], axis=0)
    return (idx.astype(np.int32, copy=False), w.astype(np.float32, copy=False)), res


def kernel(x, weight, e_score_correction_bias):
    (idx, w), _ = run_hw(x, weight, e_score_correction_bias, trace=False)
    return idx, w


# revision 20
# speedup vs baseline: 1.1949x; 1.1949x over previous
"""DeepseekV3 top-k router (moe_routing) on 8 Trainium2 NeuronCores.

Sharding (hardcoded from the problem spec):
  - Data-parallel over the token dim: 8192 tokens -> 8 shards of 1024.
  - Router weight [256, 7168] and bias [256] replicated to every core.

Numerics: logits = x.w need ~fp32 precision for the top-k indices to
match the fp32 reference exactly. Decompose x = xh + xl (fp16 hi +
residual), w = wh + wl, and compute
    logits = xh.wh  +  2^-18 * (fp8(xh).fp8(wl*2^18) + fp8(xl*2^12).fp8(wh*2^6))
with both correction terms in ONE fp8 DoubleRow matmul per k-tile
(DoubleRow contracts [K,2,M] x [K,2,N] over K and the 2-subtile dim;
measured on HW it runs at 1 cycle/output-row, its win is carrying two
fp8 products per instruction). Verified on the fixed eval inputs:
0/65536 index flips vs the fp32 reference, max weight rel err 1.3e-5.

Matmul orientation: w stationary / x moving, so each instruction covers
a whole token group (the PE re-issues LDWEIGHTS per matmul; small
instructions are dispatch/LDW-bound at ~109-135ns):
  - hh:  [K=128,128e].T @ [K=128,Nt] fp16
  - corr:[K,2,128e].T @ [K,2,Nt] fp8 DoubleRow
Logits accumulate transposed [expert, token]; hh+corr combine on
ScalarE+DVE, then 128x128 tiles transpose back via PE-transpose.

Schedule: token groups [256,256,256,128,128] with double-buffered x
tiles (no group-boundary DMA bubbles); the two trailing 128-token
groups keep the end-of-kernel epilogue tail short. Rings are static:
sync carries xh + w16, scalar carries xl8 + w8 + outputs. Every group
is chunked along k so matmuls track chunk arrival; fp8(xh) is cast
on-device on the DVE; each group's epilogue is emitted one group late
so the next group's casts never queue behind it on the DVE.
"""

import os
import sys

for _p in ("/opt/trn_rl_repo", "/root/.axon_site/_ro/trn_rl_repo"):
    if os.path.isdir(_p) and _p not in sys.path:
        sys.path.append(_p)

from contextlib import ExitStack

import numpy as np
import ml_dtypes

import concourse.bass as bass
import concourse.bacc as bacc
import concourse.mybir as mybir
import concourse.tile as tile
from concourse import masks

N_CORES = 8
T_FULL = 8192
HIDDEN = 7168
N_EXPERTS = 256
TOP_K = 8
N_GROUP = 8
TOPK_GROUP = 4
SCALING = 2.5

P = 128
GROUPS = [256, 256, 256, 256]   # token groups per core (sum 1024)
GMAX = 256
F32 = mybir.dt.float32
F16 = mybir.dt.float16
F8 = mybir.dt.float8e4
E4NP = ml_dtypes.float8_e4m3
S_XL = 2.0 ** 12              # xl8 = fp8(xl * S_XL)
S_WL = 2.0 ** 18              # wl8 = fp8(wl * S_WL)
S_WH = 2.0 ** 6               # wh8 = fp8(wh * S_WH)
S_CORR = S_XL * S_WH          # both fp8 products land at this scale
WARMUP_MMS = 120


def build_module(t_shard=T_FULL // N_CORES, hidden=HIDDEN):
    """Build + compile the per-core Bass module (SPMD: same program, 8 cores)."""
    KT = hidden // P            # hidden k-tiles (56)
    TT = t_shard // P           # token tiles per core (8)
    E = N_EXPERTS
    EPG = E // N_GROUP          # experts per group (32)
    AX = mybir.AxisListType
    OP = mybir.AluOpType
    DRM = mybir.MatmulPerfMode.DoubleRow
    starts = [sum(GROUPS[:i]) for i in range(len(GROUPS))]

    nc = bacc.Bacc("TRN2", debug=False, target_bir_lowering=False)

    # x pre-tiled per group-size class so each (group, k-chunk) DMA is a
    # plain multi-dim slice with long contiguous runs
    NG_A = sum(1 for t in GROUPS if t == 256)
    NG_B = sum(1 for t in GROUPS if t == 128)
    xh_a = nc.dram_tensor("xh16_a", [NG_A, P, KT, 256], F16, kind="ExternalInput").ap()
    xl_a = nc.dram_tensor("xl8_a", [NG_A, P, KT, 256], F8, kind="ExternalInput").ap()
    xh_b = nc.dram_tensor("xh16_b", [max(NG_B, 1), P, KT, 128], F16, kind="ExternalInput").ap()
    xl_b = nc.dram_tensor("xl8_b", [max(NG_B, 1), P, KT, 128], F8, kind="ExternalInput").ap()
    w16_in = nc.dram_tensor("w16", [P, KT, E], F16, kind="ExternalInput").ap()
    w8_in = nc.dram_tensor("w8", [P, 2, KT, E], F8, kind="ExternalInput").ap()
    bias = nc.dram_tensor("bias", [E], F32, kind="ExternalInput").ap()
    out_i = nc.dram_tensor("topk_idx", [t_shard, TOP_K], mybir.dt.int32,
                           kind="ExternalOutput").ap()
    out_w = nc.dram_tensor("topk_w", [t_shard, TOP_K], F32,
                           kind="ExternalOutput").ap()
    sink = nc.dram_tensor("warm_sink", [P, 1], F32).ap()

    kcuts = [0, 6, 20, 36, KT]
    kranges = [(kcuts[i], kcuts[i + 1]) for i in range(len(kcuts) - 1)]

    with tile.TileContext(nc) as tc, ExitStack() as ctx:
        const = ctx.enter_context(tc.tile_pool(name="const", bufs=1))
        wpool = ctx.enter_context(tc.tile_pool(name="wres", bufs=1))
        xpool = ctx.enter_context(tc.tile_pool(name="xin", bufs=2))
        cpool = ctx.enter_context(tc.tile_pool(name="cmb", bufs=2))
        spool = ctx.enter_context(tc.tile_pool(name="scr", bufs=2))
        smalls = ctx.enter_context(tc.tile_pool(name="small", bufs=2))
        opool = ctx.enter_context(tc.tile_pool(name="outs", bufs=1))
        pshh = ctx.enter_context(tc.tile_pool(name="pshh", bufs=2, space="PSUM"))
        pscc = ctx.enter_context(tc.tile_pool(name="pscc", bufs=2, space="PSUM"))
        ptp = ctx.enter_context(tc.tile_pool(name="ptp", bufs=2, space="PSUM"))
        pswarm = ctx.enter_context(tc.tile_pool(name="psw", bufs=1, space="PSUM"))

        # ---- PE warm-up: keep the HAM clock-gate busy from t=0 ----
        wu = const.tile([P, 64], F16)
        nc.vector.memset(wu[:], 0.0)
        psw = pswarm.tile([P, 64], F32)
        for _ in range(WARMUP_MMS):
            nc.tensor.matmul(psw[:64], wu[:], wu[:], start=True, stop=True)
        wsum = smalls.tile([P, 1], F32, tag="wsum")
        nc.vector.tensor_reduce(wsum[:], psw[:], axis=AX.X, op=OP.add)
        # SWDGE ring: must not block the HWDGE rings while warmup runs
        nc.gpsimd.dma_start(out=sink, in_=wsum[:])

        # ---- constants ----
        bias_bc = const.tile([P, E], F32)
        bias_src = bass.AP(tensor=bias.tensor, offset=0, ap=[[0, P], [1, E]])
        ident = const.tile([P, P], F32)
        masks.make_identity(nc, ident[:])

        # ---- resident w: fp16 hi + fp8 (wl, wh) pair ----
        w16_sb = wpool.tile([P, KT, E], F16)
        w8_sb = wpool.tile([P, 2, KT, E], F8)

        out_i_sb = opool.tile([P, TT, TOP_K], mybir.dt.int32)
        out_w_sb = opool.tile([P, TT, TOP_K], F32)

        def epilogue_tile(tt, ps_t):
            # sigmoid scores from the transposed [token, expert] PSUM tile
            s = spool.tile([P, E], F32, tag="s")
            nc.scalar.activation(s[:], ps_t[:, :E],
                                 mybir.ActivationFunctionType.Sigmoid)

            # scores for choice = sigmoid + bias
            sc = spool.tile([P, E], F32, tag="sc")
            nc.vector.tensor_tensor(sc[:], s[:], bias_bc[:], op=OP.add)

            sc_g = sc[:].rearrange("p (g c) -> p g c", c=EPG)

            # per-group top-2 sum
            gmax = smalls.tile([P, N_GROUP], F32, tag="gmax")
            nc.vector.tensor_reduce(gmax[:], sc_g, axis=AX.X, op=OP.max)
            rep = spool.tile([P, E], F32, tag="rep")
            nc.vector.match_replace(rep[:], gmax[:], sc[:], -1e30)
            gsec = smalls.tile([P, N_GROUP], F32, tag="gsec")
            nc.vector.tensor_reduce(gsec[:],
                                    rep[:].rearrange("p (g c) -> p g c", c=EPG),
                                    axis=AX.X, op=OP.max)
            gsum = smalls.tile([P, N_GROUP], F32, tag="gsum")
            nc.vector.tensor_tensor(gsum[:], gmax[:], gsec[:], op=OP.add)

            # top-4 groups: sort the 8 group scores, threshold at 4th
            gsort = smalls.tile([P, 8], F32, tag="gsort")
            nc.vector.max(gsort[:], gsum[:])
            gmask = smalls.tile([P, N_GROUP], F32, tag="gmask")
            nc.vector.tensor_scalar(gmask[:], gsum[:],
                                    gsort[:, TOPK_GROUP - 1:TOPK_GROUP], None,
                                    op0=OP.is_ge)

            # masked scores = sc * group_mask
            masked = spool.tile([P, E], F32, tag="masked")
            nc.vector.tensor_tensor(masked[:].rearrange("p (g c) -> p g c", c=EPG),
                                    sc_g,
                                    gmask[:].unsqueeze(2).broadcast_to(
                                        (P, N_GROUP, EPG)),
                                    op=OP.mult)

            # top-8 experts (desc values + indices, lax.top_k semantics)
            t8v = smalls.tile([P, TOP_K], F32, tag="t8v")
            nc.vector.max(t8v[:], masked[:])
            t8i = smalls.tile([P, TOP_K], mybir.dt.uint32, tag="t8i")
            nc.vector.max_index(t8i[:], t8v[:], masked[:])

            # output copy rides GpSimd so it stays off the DVE chain
            nc.gpsimd.tensor_copy(out_i_sb[:, tt, :], t8i[:])

            # gather sigmoid scores at the top-8 indices
            mr2 = spool.tile([P, E], F32, tag="mr2")
            nc.vector.match_replace(mr2[:], t8v[:], masked[:], -1.0)
            sel = spool.tile([P, E], F32, tag="sel")
            nc.vector.tensor_tensor(sel[:], mr2[:], masked[:], op=OP.not_equal)
            nc.vector.tensor_tensor(sel[:], sel[:], s[:], op=OP.mult)
            v8 = smalls.tile([P, TOP_K], F32, tag="v8")
            nc.vector.max(v8[:], sel[:])
            i8 = smalls.tile([P, TOP_K], mybir.dt.uint32, tag="i8")
            nc.vector.max_index(i8[:], v8[:], sel[:])
            # eqm[p, k, j] = (idx_choice[p, k] == idx_s[p, j]); sg = eqm @ v8
            eqm = smalls.tile([P, TOP_K, TOP_K], F32, tag="eqm")
            nc.vector.tensor_tensor(eqm[:],
                                    t8i[:].unsqueeze(2).broadcast_to(
                                        (P, TOP_K, TOP_K)),
                                    i8[:].unsqueeze(1).broadcast_to(
                                        (P, TOP_K, TOP_K)),
                                    op=OP.is_equal)
            nc.vector.tensor_tensor(eqm[:], eqm[:],
                                    v8[:].unsqueeze(1).broadcast_to(
                                        (P, TOP_K, TOP_K)),
                                    op=OP.mult)
            sg = smalls.tile([P, TOP_K], F32, tag="sg")
            nc.vector.tensor_reduce(sg[:], eqm[:], axis=AX.X, op=OP.add)

            # weights = sg / sum(sg) * SCALING
            den = smalls.tile([P, 1], F32, tag="den")
            nc.vector.tensor_reduce(den[:], sg[:], axis=AX.X, op=OP.add)
            rcp = smalls.tile([P, 1], F32, tag="rcp")
            nc.vector.reciprocal(rcp[:], den[:])
            nc.vector.tensor_scalar(out_w_sb[:, tt, :], sg[:], rcp[:, 0:1],
                                    SCALING, op0=OP.mult, op1=OP.mult)

        oi = out_i.rearrange("(t p) k -> p t k", p=P)
        ow = out_w.rearrange("(t p) k -> p t k", p=P)

        def epilogue_group(g, ps_h, ps_c):
            TBg = GROUPS[g]
            nt = TBg // P
            tt0 = starts[g] // P
            # combine halves: stage hh via ScalarE, stt on DVE
            sA = cpool.tile([P, 2, GMAX], F32, tag="sA")
            comb = cpool.tile([P, 2, GMAX], F32, tag="comb")
            for h in range(2):
                nc.scalar.activation(sA[:, h, :TBg], ps_h[:, h, :TBg],
                                     mybir.ActivationFunctionType.Copy)
                nc.vector.scalar_tensor_tensor(comb[:, h, :TBg], ps_c[:, h, :TBg],
                                               1.0 / S_CORR, sA[:, h, :TBg],
                                               op0=OP.mult, op1=OP.add)
            for t in range(nt):
                ps_t = ptp.tile([P, E], F32, tag="pst")
                for h in range(2):
                    nc.tensor.transpose(ps_t[:, h * P:(h + 1) * P],
                                        comb[:, h, t * P:(t + 1) * P],
                                        ident[:])
                epilogue_tile(tt0 + t, ps_t)
            nc.scalar.dma_start(out=oi[:, tt0:tt0 + nt],
                                in_=out_i_sb[:, tt0:tt0 + nt])
            nc.scalar.dma_start(out=ow[:, tt0:tt0 + nt],
                                in_=out_w_sb[:, tt0:tt0 + nt])

        pending = None
        for g, TBg in enumerate(GROUPS):
            t0 = starts[g]
            xh_t = xpool.tile([P, KT, GMAX], F16, tag="xh", name=f"xh_{g}")
            x8_t = xpool.tile([P, 2, KT, GMAX], F8, tag="x8", name=f"x8_{g}")
            gi = g if TBg == 256 else g - NG_A
            xh_src = xh_a if TBg == 256 else xh_b
            xl_src = xl_a if TBg == 256 else xl_b
            if g == 0:
                # w first, chunked, one ring per tensor (cross-ring writers
                # into one tile corrupt it): w16 leads sync, w8 leads scalar
                for (k0, k1) in kranges:
                    nc.sync.dma_start(out=w16_sb[:, k0:k1],
                                      in_=w16_in[:, k0:k1])
                    nc.scalar.dma_start(out=w8_sb[:, :, k0:k1],
                                        in_=w8_in[:, :, k0:k1])
                nc.scalar.dma_start(out=bias_bc[:], in_=bias_src)
            for (k0, k1) in kranges:
                nc.sync.dma_start(out=xh_t[:, k0:k1, :TBg],
                                  in_=xh_src[gi, :, k0:k1])
                nc.scalar.dma_start(out=x8_t[:, 1, k0:k1, :TBg],
                                    in_=xl_src[gi, :, k0:k1])
                # on-device cast x8[:,0] = fp8(xh) on the DVE
                nc.vector.tensor_copy(x8_t[:, 0, k0:k1, :TBg],
                                      xh_t[:, k0:k1, :TBg])

            ps_h = pshh.tile([P, 2, GMAX], F32, tag="psh")
            ps_c = pscc.tile([P, 2, GMAX], F32, tag="psc")
            # each expert-half runs its FULL k sweep before the other half
    # touches the same PSUM bank: accumulation groups are per-bank, so
            # the halves' groups must be consecutive and closed, never
            # interleaved. Mode flips twice per group (hh runs, then DR).
            for h in range(2):
                for k in range(KT):
                    nc.tensor.matmul(ps_h[:, h, :TBg],
                                     w16_sb[:, k, h * P:(h + 1) * P],
                                     xh_t[:, k, :TBg],
                                     start=(k == 0), stop=(k == KT - 1))
            for h in range(2):
                for k in range(KT):
                    nc.tensor.matmul(ps_c[:, h, :TBg],
                                     w8_sb[:, :, k, h * P:(h + 1) * P],
                                     x8_t[:, :, k, :TBg],
                                     start=(k == 0), stop=(k == KT - 1),
                                     perf_mode=DRM)

            # emit the PREVIOUS group's epilogue now, so this one's casts
            # did not queue behind it on the DVE
            if pending is not None:
                epilogue_group(*pending)
            pending = (g, ps_h, ps_c)
        epilogue_group(*pending)

    nc.compile()
    return nc


_CACHED = {}


def _get_module():
    key = (T_FULL // N_CORES, HIDDEN)
    if key not in _CACHED:
        _CACHED[key] = build_module(*key)
    return _CACHED[key]


def _make_in_maps(x, weight, e_score_correction_bias):
    x = np.asarray(x, dtype=np.float32)
    w = np.asarray(weight, dtype=np.float32)
    b = np.ascontiguousarray(np.asarray(e_score_correction_bias, dtype=np.float32))
    hidden = x.shape[1]
    E = w.shape[0]
    KT = hidden // P

    wT = np.ascontiguousarray(w.T)                      # [H, E] f32
    w16 = wT.astype(np.float16)
    wl8 = ((wT - w16.astype(np.float32)) * np.float32(S_WL)).astype(E4NP)
    wh8 = (w16.astype(np.float32) * np.float32(S_WH)).astype(E4NP)

    def tile_w(a):                                      # [H, E] -> [P, KT, E]
        return np.ascontiguousarray(a.reshape(KT, P, E).transpose(1, 0, 2))

    w16_t = tile_w(w16)
    w8_t = np.ascontiguousarray(
        np.stack([tile_w(wl8), tile_w(wh8)], axis=1))   # [P, 2, KT, E]

    def tile_x(a):
        # [H, T] -> dict of per-group-size stacks [NG, P, KT, TBg]
        av, bv = [], []
        t0 = 0
        for TBg in GROUPS:
            v = np.ascontiguousarray(
                a[:, t0:t0 + TBg].reshape(KT, P, TBg).transpose(1, 0, 2))
            (av if TBg == 256 else bv).append(v)
            t0 += TBg
        out = {}
        out["a"] = np.ascontiguousarray(np.stack(av)) if av else None
        out["b"] = (np.ascontiguousarray(np.stack(bv)) if bv
                    else np.zeros((1, P, KT, 128), a.dtype))
        return out

    t_shard = x.shape[0] // N_CORES
    in_maps = []
    for i in range(N_CORES):
        shardT = np.ascontiguousarray(x[i * t_shard:(i + 1) * t_shard].T)
        xh = shardT.astype(np.float16)
        xl8 = ((shardT - xh.astype(np.float32))
               * np.float32(S_XL)).astype(E4NP)
        xhp, xlp = tile_x(xh), tile_x(xl8)
        in_maps.append({"xh16_a": xhp["a"], "xl8_a": xlp["a"],
                        "xh16_b": xhp["b"], "xl8_b": xlp["b"],
                        "w16": w16_t, "w8": w8_t, "bias": b})
    return in_maps


def run_hw(x, weight, e_score_correction_bias, trace=False, **kwargs):
    """Run on the 8 NeuronCores; returns ((idx, w), BassKernelResults)."""
    from concourse.bass_utils import run_bass_kernel_spmd

    nc = _get_module()
    in_maps = _make_in_maps(x, weight, e_score_correction_bias)
    res = run_bass_kernel_spmd(nc, in_maps, core_ids=list(range(N_CORES)),
                               trace=trace, **kwargs)
    idx = np.concatenate([r["topk_idx"] for r in res.results], axis=0)
    w = np.concatenate([r["topk_w"] for r in res.results], axis=0)
    return (idx.astype(np.int32, copy=False), w.astype(np.float32, copy=False)), res


def kernel(x, weight, e_score_correction_bias):
    (idx, w), _ = run_hw(x, weight, e_score_correction_bias, trace=False)
    return idx, w
